# revision 18
# baseline (speedup 1.0000x reference)
"""Trainium2 Bass kernel: cube rasterizer + Lambertian shader.

Contract: kernel(**inputs) takes the FULL unsharded inputs (as produced by
setup_inputs()) and returns the full (rendered [480,640,4], geometry
[480,640,9]) tuple, matching reference.render().

Strategy (data-parallel over image rows, 8 cores x 60 rows):
  Every per-(pixel, triangle) quantity in the reference is affine in the NDC
  pixel coords (px, py), so each whole-tile evaluation is ONE fused op.
  For a closed CONVEX mesh the z-buffer winner at any covered pixel is the
  unique covered front-facing face (back faces lose the depth test
  everywhere except exactly on silhouette edges, where the interpolated
  attributes agree), so no per-pixel depth comparison is needed:
    - coverage per front face via the 4 outer-edge functions: ACT computes
      relu(-edge) fused into the affine eval; a pixel is covered iff the sum
      of the four relus is exactly 0.  (Identical real-valued semantics to
      the reference's per-triangle all(b>=0) union, up to the hairline
      diagonal-crack case which only differs within ~1 ulp of the diagonal.)
    - per-face position numerators N0,N1,N2 and denominator D (projective
      interpolation of world position) evaluated on the TENSOR engine as
      K=4 matmuls (px splits into even/odd partition patterns, rank 4), then
      selected per pixel with predicated copies.
    - shading: scene has normals = k*verts and uniform colors (verified at
      runtime; otherwise numpy fallback), so the interpolated normal is
      k*pos and the whole Lambert term collapses to closed form in pos:
        dot(nrm_raw, ldir) = k*(pos.lp - |pos|^2),  |ldir|^2 = |lp|^2
        - 2 pos.lp + |pos|^2.
  Device layout: each core gets 60 rows x 640 cols reshaped to a [120, 320]
  tile (partition p = 2*local_row + col_half, so py is constant per
  partition and px has rank 2 across partitions).
"""

import os
import sys

import numpy as np

if "/opt/trn_rl_repo" not in sys.path:
    sys.path.insert(0, "/opt/trn_rl_repo")

H, W = 480, 640
FOV_Y, NEAR, FAR = 40.0, 0.01, 10.0
EPS = 1e-9
NCORES = 8
ROWS = H // NCORES          # 60 rows per core
P = 2 * ROWS                # 120 partitions
F = W // 2                  # 320 free elements

f32 = np.float32
f64 = np.float64


# ----------------------------------------------------------------------------
# Host: replicate the reference vertex pipeline in float32 numpy
# ----------------------------------------------------------------------------

def _euler_matrix(angles):
    s, c = np.sin(angles.astype(f32)), np.cos(angles.astype(f32))
    sx, sy, sz = s[0], s[1], s[2]
    cx, cy, cz = c[0], c[1], c[2]
    m = np.array(
        [cy * cz, sx * sy * cz - cx * sz, cx * sy * cz + sx * sz,
         cy * sz, sx * sy * sz + cx * cz, cx * sy * sz - sx * cz,
         -sy, sx * cy, cx * cy], dtype=f32)
    return m.reshape(3, 3)


def _look_at(eye, center, world_up):
    fv = center - eye
    fv = fv / f32(np.linalg.norm(fv))
    sv = np.cross(fv, world_up).astype(f32)
    sv = sv / f32(np.linalg.norm(sv))
    uv = np.cross(sv, fv).astype(f32)
    R = np.stack([sv, uv, -fv]).astype(f32)
    t = -(R @ eye)
    top = np.concatenate([R, t[:, None]], axis=1)
    bot = np.array([[0.0, 0.0, 0.0, 1.0]], dtype=f32)
    return np.concatenate([top, bot], axis=0)


def _perspective():
    aspect = W / H
    focal = 1.0 / np.tan(np.radians(FOV_Y / 2.0))
    return np.array(
        [[focal / aspect, 0.0, 0.0, 0.0],
         [0.0, focal, 0.0, 0.0],
         [0.0, 0.0, (NEAR + FAR) / (NEAR - FAR), 2.0 * NEAR * FAR / (NEAR - FAR)],
         [0.0, 0.0, -1.0, 0.0]], dtype=f32)


def _pixel_grids():
    # bitwise-identical to the reference's f32 arithmetic
    px = (np.arange(W, dtype=f32) + f32(0.5)) / f32(W) * f32(2.0) - f32(1.0)
    py = f32(1.0) - (np.arange(H, dtype=f32) + f32(0.5)) / f32(H) * f32(2.0)
    return px, py


def _vertex_pipeline(inputs):
    euler_angles = np.asarray(inputs["euler_angles"], f32)
    cube_vertices = np.asarray(inputs["cube_vertices"], f32)
    cube_normals = np.asarray(inputs["cube_normals"], f32)
    triangles = np.asarray(inputs["triangles"], np.int64)
    eye = np.asarray(inputs["eye"], f32)
    center = np.asarray(inputs["center"], f32)
    world_up = np.asarray(inputs["world_up"], f32)

    R = _euler_matrix(euler_angles[0])
    verts_w = cube_vertices @ R.T
    normals_w = cube_normals @ R.T
    view = _look_at(eye[0], center[0], world_up[0])
    proj = _perspective()
    PV = (proj @ view).astype(f32)
    vh = np.concatenate([verts_w, np.ones((verts_w.shape[0], 1), f32)], axis=1)
    clip = vh @ PV.T

    tv = clip[triangles]                      # [T,3,4]
    x, y, z, w = tv[..., 0], tv[..., 1], tv[..., 2], tv[..., 3]
    M = np.stack([x, y, w], axis=1)           # [T,3,3]
    det = np.linalg.det(M)
    valid = np.abs(det) > EPS
    Msafe = np.where(valid[:, None, None], M, np.eye(3, dtype=f32))
    Minv = np.linalg.inv(Msafe).astype(f32)   # [T,3,3]
    zw = (z / w).astype(f32)                  # [T,3]

    return dict(R=R, verts_w=verts_w, normals_w=normals_w, clip=clip,
                triangles=triangles, M=M, det=det, valid=valid, Minv=Minv,
                zw=zw, eye=eye)


# ----------------------------------------------------------------------------
# Host: face grouping, convexity, coefficient folding
# ----------------------------------------------------------------------------

def _group_faces(verts_w, triangles, valid):
    """Group triangles into planar faces; orient outward; check convexity.
    Returns None if the mesh is not a closed convex polyhedron (within tol)."""
    V = verts_w.astype(f64)
    T = triangles.shape[0]
    centroid = V.mean(axis=0)
    tol = 1e-5 * max(1.0, np.abs(V).max())

    keys = []
    for t in range(T):
        v0, v1, v2 = V[triangles[t]]
        n = np.cross(v1 - v0, v2 - v0)
        ln = np.linalg.norm(n)
        if ln < 1e-12:
            keys.append(None)
            continue
        n = n / ln
        d = n @ v0
        if n @ centroid > d:          # orient outward (centroid inside)
            n, d = -n, -d
        keys.append((n, d))

    faces = []
    for t in range(T):
        if keys[t] is None:
            continue
        n, d = keys[t]
        placed = False
        for fc in faces:
            n2, d2 = fc["plane"]
            if np.abs(n @ n2 - 1.0) < 1e-6 and abs(d - d2) < tol:
                fc["tris"].append(t)
                placed = True
                break
        if not placed:
            faces.append({"plane": (n, d), "tris": [t]})

    for fc in faces:
        n, d = fc["plane"]
        if (V @ n - d).max() > tol:
            return None
    return faces


def _precompute(inputs):
    vp = _vertex_pipeline(inputs)
    verts_w, normals_w = vp["verts_w"], vp["normals_w"]
    triangles, valid, Minv = vp["triangles"], vp["valid"], vp["Minv"]
    eye = vp["eye"][0].astype(f64)

    colors = np.asarray(inputs["vertex_diffuse_colors"], f32)[0]   # [V,3]
    light_pos = np.asarray(inputs["light_positions"], f32)[0]      # [L,3]
    light_int = np.asarray(inputs["light_intensities"], f32)[0]    # [L,3]

    ok = True
    faces = _group_faces(verts_w, triangles, valid)
    if faces is None:
        ok = False

    denom = float((verts_w.astype(f64) ** 2).sum())
    knrm = float((normals_w.astype(f64) * verts_w.astype(f64)).sum() / denom) \
        if denom > 0 else 0.0
    if not np.allclose(normals_w, f32(knrm) * verts_w, atol=2e-6):
        ok = False

    cbar = colors[0].copy()
    if not np.all(colors == cbar[None, :]):
        ok = False

    if not ok:
        return None

    # front faces with at least one valid triangle
    front_faces = []
    for fc in faces:
        members = [t for t in fc["tris"] if valid[t]]
        if not members:
            continue
        n, d = fc["plane"]
        if (n @ eye - d) > 0.0:
            front_faces.append({"members": members})
    K = len(front_faces)
    if K == 0:
        return None

    Minv64 = Minv.astype(f64)
    V64 = verts_w.astype(f64)

    # ---- coverage (relu) edge functions: list of (c0,c1,c2) f64 rows, and
    #      per-face the slice of edge indices
    edge_rows = []
    face_edges = []
    for fc in front_faces:
        mem = fc["members"]
        rows = []
        if len(mem) == 2:
            vA = list(triangles[mem[0]])
            vB = list(triangles[mem[1]])
            shared = [v for v in vA if v in vB]
            if len(shared) == 2:
                for t, vl in ((mem[0], vA), (mem[1], vB)):
                    for sv in shared:
                        rows.append(Minv64[t][vl.index(sv)])
            else:                       # unexpected: fall back to per-tri
                for t in mem:
                    rows.extend(Minv64[t])
        else:
            for t in mem:
                rows.extend(Minv64[t])
        face_edges.append((len(edge_rows), len(rows)))
        edge_rows.extend(rows)
    NEDGE = len(edge_rows)
    Cedge = np.stack(edge_rows)                 # [NEDGE, 3] f64

    # ---- face plane (PE) functions: N0,N1,N2,D per face
    Cface = np.zeros((4 * K, 3), dtype=f64)
    for fi, fc in enumerate(front_faces):
        rep = fc["members"][0]
        A = Minv64[rep]
        Vt = V64[triangles[rep]]
        Cface[4 * fi:4 * fi + 3, :] = Vt.T @ A
        Cface[4 * fi + 3, :] = A.sum(axis=0)

    # ---- per-core tensors
    px32, py32 = _pixel_grids()
    PX = np.empty((P, F), dtype=f32)
    PX[0::2, :] = px32[:F][None, :]
    PX[1::2, :] = px32[F:][None, :]

    # relu-eval scales/biases (negated)
    SCLR = np.tile((-Cedge[:, 0]).astype(f32)[None, :], (P, 1))   # [P, NEDGE]
    BIAR = []
    for k in range(NCORES):
        rows = np.arange(ROWS) + k * ROWS
        pyk = py32[rows].astype(f64)
        pyp = np.repeat(pyk, 2)                # [120]
        b = (-(Cedge[None, :, 1] * pyp[:, None] + Cedge[None, :, 2])).astype(f32)
        BIAR.append(np.ascontiguousarray(b))

    # PE operands: lhsT [4, P] rows (even, odd, py, 1); rhs [4, 320*4K]
    LHS = []
    for k in range(NCORES):
        rows = np.arange(ROWS) + k * ROWS
        pyp = np.repeat(py32[rows], 2).astype(f32)
        l = np.zeros((4, P), dtype=f32)
        l[0, 0::2] = 1.0
        l[1, 1::2] = 1.0
        l[2, :] = pyp
        l[3, :] = 1.0
        LHS.append(np.ascontiguousarray(l))
    NF = 4 * K
    RHS = np.zeros((4, F * NF), dtype=f32)
    pxe64 = px32[:F].astype(f64)
    pxo64 = px32[F:].astype(f64)
    for j in range(NF):
        c0, c1, c2 = Cface[j]
        RHS[0, j * F:(j + 1) * F] = (c0 * pxe64).astype(f32)
        RHS[1, j * F:(j + 1) * F] = (c0 * pxo64).astype(f32)
        RHS[2, j * F:(j + 1) * F] = f32(c1)
        RHS[3, j * F:(j + 1) * F] = f32(c2)

    lp64 = light_pos.astype(f64)
    return dict(
        K=K, NEDGE=NEDGE, face_edges=face_edges,
        PX=PX, SCLR=np.ascontiguousarray(SCLR), BIAR=BIAR,
        LHS=LHS, RHS=np.ascontiguousarray(RHS), Cface=Cface, Cedge=Cedge,
        knrm=f32(knrm), cbar=cbar.astype(f32),
        light_pos=light_pos, light_int=light_int,
        lp_sq=[f32(v) for v in (lp64 ** 2).sum(axis=1)],
        unit_shading=bool(np.all(light_int == 1.0) and np.all(cbar == 1.0)),
    )


# ----------------------------------------------------------------------------
# Numpy simulation of the exact device program (for validation/debug)
# ----------------------------------------------------------------------------

def _simulate_core(pre, k):
    PX, SCLR, BIAR = pre["PX"], pre["SCLR"], pre["BIAR"][k]
    K = pre["K"]
    knrm, cbar = pre["knrm"], pre["cbar"]
    lp, li = pre["light_pos"], pre["light_int"]
    L = lp.shape[0]

    # coverage
    relu = np.maximum(PX * SCLR[:, None] if False else 0, 0)  # placeholder
    sface = []
    for fi in range(K):
        o, n = pre["face_edges"][fi]
        s = None
        for j in range(o, o + n):
            r = np.maximum(PX * SCLR[:, j:j + 1] + BIAR[:, j:j + 1], f32(0.0))
            s = r if s is None else s + r
        sface.append(s)
    win = [(s == 0.0) for s in sface]
    hp = sface[0]
    for s in sface[1:]:
        hp = hp * s
    hitf = (hp == 0.0).astype(f32)

    # face planes (PE emulation: affine eval in f32)
    LHS, RHS = pre["LHS"][k], pre["RHS"]
    def pe_eval(j):
        # emulate K=4 fp32 matmul accumulation
        acc = np.zeros((P, F), f32)
        for kk in range(4):
            acc = acc + LHS[kk][:, None] * RHS[kk, j * F:(j + 1) * F][None, :]
        return acc
    N0 = np.zeros((P, F), f32)
    N1 = np.zeros((P, F), f32)
    N2 = np.zeros((P, F), f32)
    Dv = np.ones((P, F), f32)
    for fi in range(K):
        N0 = np.where(win[fi], pe_eval(4 * fi), N0)
        N1 = np.where(win[fi], pe_eval(4 * fi + 1), N1)
        N2 = np.where(win[fi], pe_eval(4 * fi + 2), N2)
        Dv = np.where(win[fi], pe_eval(4 * fi + 3), Dv)

    rD = (f32(1.0) / Dv).astype(f32)
    pos = [N0 * rD, N1 * rD, N2 * rD]
    sq = [p * p for p in pos]
    q = sq[0] + sq[1] + sq[2]
    npos = np.sqrt(q * f32(knrm) * f32(knrm)) + f32(EPS)

    dif = [None, None, None]
    for l in range(L):
        t0 = pos[0] * lp[l, 0]
        u = pos[1] * lp[l, 1] + t0
        u = pos[2] * lp[l, 2] + u
        dotpl = u - q
        qplus = q + pre["lp_sq"][l]
        d2 = u * f32(-2.0) + qplus
        nl = np.sqrt(d2) + f32(EPS)
        den = npos * nl
        rden = (f32(1.0) / den).astype(f32)
        t = dotpl * rden
        lam = np.maximum(t * f32(knrm), f32(0.0))
        for c in range(3):
            v = lam * li[l, c]
            dif[c] = v if dif[c] is None else dif[c] + v

    rend = np.empty((4, P, F), f32)
    geo = np.empty((9, P, F), f32)
    for c in range(3):
        colm = cbar[c] * hitf
        rend[c] = dif[c] * colm
        geo[c] = pos[c] * hitf
        geo[3 + c] = (pos[c] * knrm) * hitf
        geo[6 + c] = colm
    rend[3] = hitf
    return rend, geo


def _assemble(per_core_rend, per_core_geo):
    rend = np.empty((H, W, 4), f32)
    geo = np.empty((H, W, 9), f32)
    for k in range(NCORES):
        r = per_core_rend[k].reshape(4, ROWS, 2, F).transpose(1, 2, 3, 0)
        g = per_core_geo[k].reshape(9, ROWS, 2, F).transpose(1, 2, 3, 0)
        rend[k * ROWS:(k + 1) * ROWS] = r.reshape(ROWS, W, 4)
        geo[k * ROWS:(k + 1) * ROWS] = g.reshape(ROWS, W, 9)
    return rend, geo


def kernel_numpy_sim(**inputs):
    """Full-pipeline numpy emulation of the device program (no hardware)."""
    pre = _precompute(inputs)
    if pre is None:
        return _numpy_fallback(inputs)
    outs = [_simulate_core(pre, k) for k in range(NCORES)]
    return _assemble([o[0] for o in outs], [o[1] for o in outs])


# ----------------------------------------------------------------------------
# Numpy fallback (faithful reference reimplementation; only used if the
# inputs are not the convex fast-path scene)
# ----------------------------------------------------------------------------

def _numpy_fallback(inputs):
    vp = _vertex_pipeline(inputs)
    verts_w, normals_w = vp["verts_w"], vp["normals_w"]
    triangles, valid, Minv, zw = vp["triangles"], vp["valid"], vp["Minv"], vp["zw"]
    colors = np.asarray(inputs["vertex_diffuse_colors"], f32)[0]
    lp = np.asarray(inputs["light_positions"], f32)[0]
    li = np.asarray(inputs["light_intensities"], f32)[0]

    px, py = _pixel_grids()
    Pg = np.stack(np.broadcast_arrays(px[None, :], py[:, None],
                                      np.ones((1, 1), f32)), axis=-1)
    b = np.einsum("tij,hwj->thwi", Minv, Pg).astype(f32)
    inside = np.all(b >= 0.0, axis=-1) & valid[:, None, None]
    bsum = np.sum(b, axis=-1)
    bn = b / np.where(np.abs(bsum) > EPS, bsum, f32(1.0))[..., None]
    depth = np.einsum("thwi,ti->thw", bn, zw).astype(f32)
    depth = np.where(inside, depth, f32(1e9))
    tri_id = np.argmin(depth, axis=0)
    hit = np.any(inside, axis=0)
    bn_pix = np.take_along_axis(bn, tri_id[None, ..., None], axis=0)[0]
    attrs = np.concatenate([verts_w, normals_w, colors], axis=-1)
    tri_attrs = attrs[triangles]
    pix_attrs = np.einsum("hwi,hwic->hwc", bn_pix, tri_attrs[tri_id]).astype(f32)
    pos = pix_attrs[..., 0:3]
    nrm = pix_attrs[..., 3:6]
    nrm = nrm / (np.linalg.norm(nrm, axis=-1, keepdims=True) + f32(EPS))
    col = pix_attrs[..., 6:9]
    ldir = lp[None, None] - pos[..., None, :]
    ldir = ldir / (np.linalg.norm(ldir, axis=-1, keepdims=True) + f32(EPS))
    lam = np.maximum(np.sum(nrm[..., None, :] * ldir, axis=-1), 0.0).astype(f32)
    diffuse = np.einsum("hwl,lc->hwc", lam, li).astype(f32)
    mask = hit.astype(f32)[..., None]
    rgb = diffuse * col * mask
    rendered = np.concatenate([rgb, mask], axis=-1)
    geometry = pix_attrs * mask
    return rendered, geometry


# ----------------------------------------------------------------------------
# Bass program
# ----------------------------------------------------------------------------

_BUILD_CACHE = {}


def _build_key(pre):
    struct = (pre["K"], pre["NEDGE"], tuple(pre["face_edges"]))
    scal = (float(pre["knrm"]), tuple(map(float, pre["cbar"])),
            tuple(map(float, pre["light_pos"].ravel())),
            tuple(map(float, pre["light_int"].ravel())))
    return (struct, scal)


def _build_bass(pre):
    import concourse.bacc as bacc
    import concourse.mybir as mybir
    from concourse.tile import TileContext

    AL = mybir.AluOpType
    AF = mybir.ActivationFunctionType
    dt = mybir.dt.float32

    K, NEDGE = pre["K"], pre["NEDGE"]
    NF = 4 * K
    knrm = float(pre["knrm"])
    cbar = pre["cbar"]
    lp, li = pre["light_pos"], pre["light_int"]
    L = lp.shape[0]
    unit = pre["unit_shading"]

    nc = bacc.Bacc("TRN2", target_bir_lowering=False)
    cst_d = nc.dram_tensor("cst", [P, F + 2 * NEDGE], dt, kind="ExternalInput")
    lhs_d = nc.dram_tensor("lhs", [4, P], dt, kind="ExternalInput")
    rhs_d = nc.dram_tensor("rhs", [4, F * NF], dt, kind="ExternalInput")
    # compact outputs: [rgbbase(1 or 3), hitf] and [geoP x3, geoN x3]
    nrgb = 1 if (unit and L == 1) else 3
    rend_d = nc.dram_tensor("rend", [nrgb + 1, P, F], dt, kind="ExternalOutput")
    geo_d = nc.dram_tensor("geo", [6, P, F], dt, kind="ExternalOutput")

    with TileContext(nc) as tc:
        with tc.tile_pool(name="main", bufs=1) as pool, \
             tc.tile_pool(name="ps", bufs=1, space="PSUM") as psp:
            cst = pool.tile([P, F + 2 * NEDGE], dt, name="cst_t")
            nc.sync.dma_start(cst, cst_d[:, :])
            px = cst[:, 0:F]
            sclr = cst[:, F:F + NEDGE]
            biar = cst[:, F + NEDGE:F + 2 * NEDGE]
            lhs = pool.tile([4, P], dt, name="lhs_t")
            nc.scalar.dma_start(lhs, lhs_d[:, :])
            rhs = pool.tile([4, F * NF], dt, name="rhs_t")
            nc.scalar.dma_start(rhs, rhs_d[:, :])

            # ---- coverage: relu(-edge) on ACT, per-face sums on Pool
            sface = []
            for fi in range(K):
                o, n = pre["face_edges"][fi]
                rs = []
                for j in range(o, o + n):
                    r = pool.tile([P, F], dt, name=f"r_{fi}_{j}")
                    nc.scalar.activation(r, px, AF.Relu,
                                         bias=biar[:, j:j + 1],
                                         scale=sclr[:, j:j + 1])
                    rs.append(r)
                s01 = pool.tile([P, F], dt, name=f"s01_{fi}")
                nc.gpsimd.tensor_tensor(s01, rs[0], rs[1], AL.add)
                s23 = pool.tile([P, F], dt, name=f"s23_{fi}")
                if len(rs) == 4:
                    nc.gpsimd.tensor_tensor(s23, rs[2], rs[3], AL.add)
                elif len(rs) == 3:
                    nc.gpsimd.tensor_scalar(s23, rs[2], 0.0, None, AL.add)
                else:
                    nc.gpsimd.memset(s23, 0.0)
                sf = pool.tile([P, F], dt, name=f"sf_{fi}")
                nc.gpsimd.tensor_tensor(sf, s01, s23, AL.add)
                sface.append(sf)

            wins = []
            for fi in range(K):
                w = pool.tile([P, F], mybir.dt.uint8, name=f"win_{fi}")
                nc.gpsimd.tensor_scalar(w, sface[fi], 0.0, None, AL.is_equal)
                wins.append(w)

            # hit = any face covered  <=>  prod(sface) == 0
            hp = sface[0]
            for fi in range(1, K):
                nhp = pool.tile([P, F], dt, name=f"hp_{fi}")
                nc.gpsimd.tensor_tensor(nhp, hp, sface[fi], AL.mult)
                hp = nhp
            hitf = pool.tile([P, F], dt, name="hitf")
            nc.gpsimd.tensor_scalar(hitf, hp, 0.0, None, AL.is_equal)

            # ---- face plane planes on PE -> PSUM, select via copy_predicated
            N0a = pool.tile([P, F], dt, name="N0a")
            N1a = pool.tile([P, F], dt, name="N1a")
            N2a = pool.tile([P, F], dt, name="N2a")
            Da = pool.tile([P, F], dt, name="Da")
            nc.gpsimd.memset(N0a, 0.0)
            nc.gpsimd.memset(N1a, 0.0)
            nc.gpsimd.memset(N2a, 0.0)
            nc.gpsimd.memset(Da, 1.0)

            accs = (N0a, N1a, N2a, Da)
            for fi in range(K):
                for c in range(4):
                    j = 4 * fi + c
                    pt = psp.tile([P, F], dt, name=f"pe_{fi}_{c}",
                                  tag="pe", bufs=8)
                    nc.tensor.matmul(pt, lhs[:, :], rhs[:, j * F:(j + 1) * F],
                                     start=True, stop=True)
                    nc.vector.copy_predicated(accs[c], wins[fi], pt)

            # ---- epilogue
            rD = pool.tile([P, F], dt, name="rD")
            scr = pool.tile([P, F], dt, name="scr")
            nc.vector.reciprocal_approx_accurate(rD, Da, scr)

            pos = []
            for c, Nc in enumerate((N0a, N1a, N2a)):
                pc = pool.tile([P, F], dt, name=f"pos{c}")
                nc.vector.tensor_tensor(pc, Nc, rD, AL.mult)
                pos.append(pc)

            sq = []
            for c in range(3):
                s = pool.tile([P, F], dt, name=f"sq{c}")
                nc.scalar.activation(s, pos[c], AF.Square)
                sq.append(s)
            q01 = pool.tile([P, F], dt, name="q01")
            nc.gpsimd.tensor_tensor(q01, sq[0], sq[1], AL.add)
            q = pool.tile([P, F], dt, name="q")
            nc.gpsimd.tensor_tensor(q, q01, sq[2], AL.add)

            npos = pool.tile([P, F], dt, name="npos")
            nc.scalar.activation(npos, q, AF.Sqrt, bias=0.0,
                                 scale=float(f32(knrm) * f32(knrm)))
            npe = pool.tile([P, F], dt, name="npe")
            nc.gpsimd.tensor_scalar(npe, npos, float(f32(EPS)), None, AL.add)

            dif = [None, None, None]
            for l in range(L):
                t0 = pool.tile([P, F], dt, name=f"t0_{l}")
                nc.gpsimd.tensor_scalar(t0, pos[0], float(lp[l, 0]), None,
                                        AL.mult)
                u01 = pool.tile([P, F], dt, name=f"u01_{l}")
                nc.vector.scalar_tensor_tensor(u01, pos[1], float(lp[l, 1]),
                                               t0, AL.mult, AL.add)
                u = pool.tile([P, F], dt, name=f"u_{l}")
                nc.vector.scalar_tensor_tensor(u, pos[2], float(lp[l, 2]),
                                               u01, AL.mult, AL.add)
                dotpl = pool.tile([P, F], dt, name=f"dotpl_{l}")
                nc.vector.tensor_tensor(dotpl, u, q, AL.subtract)
                qplus = pool.tile([P, F], dt, name=f"qplus_{l}")
                nc.gpsimd.tensor_scalar(qplus, q, float(pre["lp_sq"][l]), None,
                                        AL.add)
                d2 = pool.tile([P, F], dt, name=f"d2_{l}")
                nc.vector.scalar_tensor_tensor(d2, u, -2.0, qplus,
                                               AL.mult, AL.add)
                nl = pool.tile([P, F], dt, name=f"nl_{l}")
                nc.scalar.activation(nl, d2, AF.Sqrt)
                nle = pool.tile([P, F], dt, name=f"nle_{l}")
                nc.gpsimd.tensor_scalar(nle, nl, float(f32(EPS)), None, AL.add)
                den = pool.tile([P, F], dt, name=f"den_{l}")
                nc.gpsimd.tensor_tensor(den, npe, nle, AL.mult)
                rden = pool.tile([P, F], dt, name=f"rden_{l}")
                scr2 = pool.tile([P, F], dt, name=f"scr2_{l}")
                nc.vector.reciprocal_approx_accurate(rden, den, scr2)
                t = pool.tile([P, F], dt, name=f"t_{l}")
                nc.gpsimd.tensor_tensor(t, dotpl, rden, AL.mult)
                lam = pool.tile([P, F], dt, name=f"lam_{l}")
                nc.scalar.activation(lam, t, AF.Relu, bias=0.0, scale=knrm)
                if unit and L == 1:
                    dif = [lam, lam, lam]
                else:
                    for c in range(3):
                        v = pool.tile([P, F], dt, name=f"dv_{l}_{c}")
                        nc.gpsimd.tensor_scalar(v, lam, float(li[l, c]), None,
                                                AL.mult)
                        if dif[c] is None:
                            dif[c] = v
                        else:
                            acc = pool.tile([P, F], dt, name=f"da_{l}_{c}")
                            nc.vector.tensor_tensor(acc, dif[c], v, AL.add)
                            dif[c] = acc

            # rgb planes: col_c = cbar_c * bnsum with bnsum = D*(1/D) == 1.0
            # up to 1-2 ulp, so rgb_c = dif_c * cbar_c * hitf.
            if unit and L == 1:
                rb = pool.tile([P, F], dt, name="rgbbase")
                nc.vector.tensor_tensor(rb, dif[0], hitf, AL.mult)
                nc.sync.dma_start(rend_d[0], rb)
            else:
                for c in range(3):
                    cm = pool.tile([P, F], dt, name=f"colm{c}")
                    nc.gpsimd.tensor_scalar(cm, hitf, float(cbar[c]), None,
                                            AL.mult)
                    rgb = pool.tile([P, F], dt, name=f"rgb{c}")
                    nc.vector.tensor_tensor(rgb, dif[c], cm, AL.mult)
                    nc.sync.dma_start(rend_d[c], rgb)
            nc.scalar.dma_start(rend_d[nrgb], hitf)

            gp_engines = [nc.gpsimd, nc.gpsimd, nc.scalar]
            gn_engines = [nc.scalar, nc.sync, nc.gpsimd]
            for c in range(3):
                gp = pool.tile([P, F], dt, name=f"geoP{c}")
                nc.gpsimd.tensor_tensor(gp, pos[c], hitf, AL.mult)
                gp_engines[c].dma_start(geo_d[c], gp)
            for c in range(3):
                gn = pool.tile([P, F], dt, name=f"geoN{c}")
                nc.vector.scalar_tensor_tensor(gn, pos[c], knrm, hitf,
                                               AL.mult, AL.mult)
                gn_engines[c].dma_start(geo_d[3 + c], gn)

    nc.compile()
    return nc


# ----------------------------------------------------------------------------
# Entry point
# ----------------------------------------------------------------------------

def kernel(**inputs):
    pre = _precompute(inputs)
    if pre is None:
        return _numpy_fallback(inputs)

    from concourse.bass_utils import run_bass_kernel_spmd

    key = _build_key(pre)
    cached = _BUILD_CACHE.get(key)
    if cached is None:
        cached = _build_bass(pre)
        _BUILD_CACHE[key] = cached
    nc = cached

    in_maps = [
        {"cst": np.ascontiguousarray(
            np.concatenate([pre["PX"], pre["SCLR"], pre["BIAR"][k]], axis=1)),
         "lhs": pre["LHS"][k],
         "rhs": pre["RHS"]}
        for k in range(NCORES)
    ]
    res = run_bass_kernel_spmd(
        nc, in_maps, core_ids=list(range(NCORES)),
        trace=bool(int(os.environ.get("KERNEL_TRACE", "0"))),
    )
    cbar = pre["cbar"]
    rends, geos = [], []
    for k in range(NCORES):
        rc = res.results[k]["rend"]          # [nrgb+1, P, F]
        gc = res.results[k]["geo"]           # [6, P, F]
        hit = rc[-1]
        rfull = np.empty((4, P, F), f32)
        if rc.shape[0] == 2:                 # unit shading: one rgb plane
            rfull[0] = rc[0]
            rfull[1] = rc[0]
            rfull[2] = rc[0]
        else:
            rfull[0:3] = rc[0:3]
        rfull[3] = hit
        gfull = np.empty((9, P, F), f32)
        gfull[0:6] = gc
        for c in range(3):
            gfull[6 + c] = cbar[c] * hit
        rends.append(rfull)
        geos.append(gfull)
    out = _assemble(rends, geos)
    kernel.last_results = res
    return out
